# revision 1
# baseline (speedup 1.0000x reference)
"""GateRow kernel for Trainium2 (8 NeuronCores, SPMD data-parallel over batch).

Problem: out[b, g] = gates[g, 2*x[b, c0[g]] + x[b, c1[g]]]
  x: [16384, 8192] bool, gates: [8192, 4] bool, choices: [8192, 2] int32.

Strategy (per core, batch-sharded BS=2048):
  host:  build a doubled lookup table TAB = [x^T ; ~x^T ; ones ; zeros]
         (uint8, one row per input wire, BS bytes per row).  Classify each
         gate's 4-entry truth table into  out = (s>=t1) ^ (s>=t2)  with
         s = va + vb, where va/vb are the (possibly inverted / constant)
         gathered operand rows.  This covers all 16 boolean functions.
  device:
    1. dma_gather rows of TAB -> operand tiles [128 gates, BS] uint8
    2. one fused custom-DVE pass: l = (a+b >= t1) ^ (a+b >= t2) -> bf16
    3. PE transpose (identity matmul) [128,128] tiles -> PSUM f32
    4. ACT copies PSUM -> SBUF uint8 (cast)
    5. DMA out rows [b, g] (contiguous per batch row)
"""

import sys

for _p in ("/opt/trn_rl_repo", "/opt/pypackages"):
    if _p not in sys.path:
        sys.path.append(_p)

from contextlib import ExitStack

import numpy as np
import ml_dtypes

import concourse.bass as bass
import concourse.bacc as bacc
import concourse.tile as tile
import concourse.mybir as mybir
from concourse.bass_utils import run_bass_kernel_spmd

B, N, G, NCORES = 16384, 8192, 8192, 8
BS = B // NCORES  # 2048 batch rows per core

# ---------------------------------------------------------------------------
# Gate classification: truth table (4 bits, bit (2a+b)) ->
#   (fa, fb, t1, t2) with fa/fb in {0: v, 1: ~v, 2: one, 3: zero}
#   such that f(a,b) == ((va+vb) >= t1) ^ ((va+vb) >= t2)
# ---------------------------------------------------------------------------


def _classify_gates():
    forms = np.zeros((16, 4), dtype=np.int64)
    for tt in range(16):
        found = False
        for fa in range(4):
            for fb in range(4):
                for t1 in range(4):
                    for t2 in range(4):
                        ok = True
                        for a in (0, 1):
                            for b in (0, 1):
                                va = (a, 1 - a, 1, 0)[fa]
                                vb = (b, 1 - b, 1, 0)[fb]
                                s = va + vb
                                v = int(s >= t1) ^ int(s >= t2)
                                if v != ((tt >> (2 * a + b)) & 1):
                                    ok = False
                        if ok and not found:
                            forms[tt] = (fa, fb, t1, t2)
                            found = True
        assert found, f"truth table {tt} not representable"
    return forms


_FORMS = _classify_gates()

# ---------------------------------------------------------------------------
# Custom DVE op:  out = ((in0+in1) >= s0) ^ ((in0+in1) >= s1)
# ---------------------------------------------------------------------------

_GATE_LUT_OP = None


def _register_gate_lut():
    global _GATE_LUT_OP
    if _GATE_LUT_OP is not None:
        return _GATE_LUT_OP
    import concourse.dve_ops as dve_ops_mod
    from concourse.dve_ops import DveOp
    from concourse.dve_spec import Spec, Src0, Src1, C0, C1, lower, _has_src1
    from concourse.dve_uop import DveOpSpec

    name = "GATE_LUT_ANT"
    if any(op.name == name for op in dve_ops_mod.OPS):
        _GATE_LUT_OP = next(op for op in dve_ops_mod.OPS if op.name == name)
        return _GATE_LUT_OP

    s = Src0 + Src1
    spec = Spec(
        body=(s >= C0) ^ (s >= C1),
        reference=lambda in0, in1, s0, s1, imm2: (
            ((in0 + in1) >= s0) != ((in0 + in1) >= s1)
        ).astype(np.float32),
    )
    row = dve_ops_mod._CUSTOM_DVE_ROW_BASE + len(dve_ops_mod.OPS)
    dve_ops_mod._SUB_OPCODE_FOR_NAME[name] = row
    shas = {}
    for ver in ("v3", "v4"):
        uops = lower(spec, ver=ver)
        shas[ver] = DveOpSpec(
            name=name, opcode=row, uops=uops, rd1_en=_has_src1(spec)
        ).sha(ver)
    op = DveOp(name, spec, subdim=False, uops_sha=shas)
    dve_ops_mod.OPS.append(op)
    dve_ops_mod.CUSTOM_DVE_SPECS[name] = spec
    _GATE_LUT_OP = op
    return op


# ---------------------------------------------------------------------------
# Device program builder (parameterized so a small version can be simulated)
# ---------------------------------------------------------------------------


def build_nc(bs=BS, n=N, g=G, group=8, ncores=NCORES):
    """One SPMD program; all cores run it on their own batch shard."""
    lut_op = _register_gate_lut()
    nblk = g // 128          # gate blocks of 128
    ngrp = nblk // group     # gather groups
    ntab = 2 * n + 2         # x^T rows, ~x^T rows, ones row, zeros row
    mtiles = bs // 128       # batch sub-tiles per core
    nidx = group * 128       # indices per dma_gather call
    percall = nidx // 16     # int16s per partition per call

    nc = bacc.Bacc(
        "TRN2", target_bir_lowering=False, debug=False, num_devices=ncores
    )
    tab = nc.dram_tensor("tab", [ntab, bs], mybir.dt.uint8, kind="ExternalInput")
    idxs = nc.dram_tensor(
        "idxs", [128, 2 * ngrp * percall], mybir.dt.int16, kind="ExternalInput"
    )
    cst = nc.dram_tensor("cst", [128, 2 * nblk], mybir.dt.float32, kind="ExternalInput")
    ident = nc.dram_tensor("ident", [128, 128], mybir.dt.bfloat16, kind="ExternalInput")
    outd = nc.dram_tensor("out", [bs, g], mybir.dt.uint8, kind="ExternalOutput")

    with tile.TileContext(nc) as tc, ExitStack() as ctx:
        pconst = ctx.enter_context(tc.tile_pool(name="const", bufs=1))
        pgather = ctx.enter_context(tc.tile_pool(name="gather", bufs=2))
        pl = ctx.enter_context(tc.tile_pool(name="lut", bufs=2))
        posb = ctx.enter_context(tc.tile_pool(name="osb", bufs=2))
        pps = ctx.enter_context(tc.tile_pool(name="ps", bufs=4, space="PSUM"))

        idx_t = pconst.tile([128, idxs.shape[1]], mybir.dt.int16)
        nc.sync.dma_start(idx_t[:], idxs[:])
        cst_t = pconst.tile([128, 2 * nblk], mybir.dt.float32)
        nc.sync.dma_start(cst_t[:], cst[:])
        id_t = pconst.tile([128, 128], mybir.dt.bfloat16)
        nc.sync.dma_start(id_t[:], ident[:])

        for gi in range(ngrp):
            a_t = pgather.tile([128, group, bs], mybir.dt.uint8, tag="a")
            b_t = pgather.tile([128, group, bs], mybir.dt.uint8, tag="b")
            off = gi * 2 * percall
            nc.gpsimd.dma_gather(
                a_t[:],
                tab[:],
                idx_t[:, off : off + percall],
                nidx,
                nidx,
                bs,
                single_packet=False,
            )
            nc.gpsimd.dma_gather(
                b_t[:],
                tab[:],
                idx_t[:, off + percall : off + 2 * percall],
                nidx,
                nidx,
                bs,
                single_packet=False,
            )
            ls = []
            for j in range(group):
                bk = gi * group + j
                l_t = pl.tile([128, bs], mybir.dt.bfloat16, tag=f"l{j}")
                nc.vector._custom_dve(
                    lut_op,
                    out=l_t[:],
                    in0=a_t[:, j, :],
                    in1=b_t[:, j, :],
                    s0=cst_t[:, bk : bk + 1],
                    s1=cst_t[:, nblk + bk : nblk + bk + 1],
                )
                ls.append(l_t)
            for m in range(mtiles):
                osb = posb.tile([128, group * 128], mybir.dt.uint8, tag=f"o{m}")
                ps = pps.tile([128, group * 128], mybir.dt.bfloat16)
                for j in range(group):
                    nc.tensor.transpose(
                        ps[:, j * 128 : (j + 1) * 128],
                        ls[j][:, m * 128 : (m + 1) * 128],
                        id_t[:],
                    )
                nc.scalar.activation(
                    osb[:], ps[:], mybir.ActivationFunctionType.Copy
                )
                nc.sync.dma_start(
                    outd[
                        m * 128 : (m + 1) * 128,
                        gi * group * 128 : (gi + 1) * group * 128,
                    ],
                    osb[:],
                )
    nc.compile()
    return nc


# ---------------------------------------------------------------------------
# Host-side input prep
# ---------------------------------------------------------------------------


def _prep_inputs(x, gates, choices, bs=BS, n=N, g=G, group=8, ncores=NCORES):
    nblk = g // 128
    ngrp = nblk // group
    x8 = np.asarray(x, dtype=np.uint8)
    gates8 = np.asarray(gates, dtype=np.uint8)
    ch = np.asarray(choices, dtype=np.int64)

    tt = (gates8 << np.arange(4, dtype=np.uint8)).sum(axis=1).astype(np.int64)
    fa, fb, t1, t2 = (_FORMS[tt, k] for k in range(4))

    # operand row index in TAB for each gate
    ia = np.where(fa <= 1, ch[:, 0] + fa * n, 2 * n + (fa - 2))
    ib = np.where(fb <= 1, ch[:, 1] + fb * n, 2 * n + (fb - 2))
    assert ia.max() < 2 * n + 2 and ib.max() < 2 * n + 2

    # dma_gather wrapped index layout: per call, idx i -> partition i%16,
    # slot i//16; replicated across the 8 gpsimd cores (x8 partitions).
    cols = []
    for gi in range(ngrp):
        for arr in (ia, ib):
            flat = arr[gi * group * 128 : (gi + 1) * group * 128].astype(np.int16)
            wrapped = flat.reshape(-1, 16).T  # [16, nidx/16]
            cols.append(np.tile(wrapped, (8, 1)))  # [128, nidx/16]
    idxs_np = np.ascontiguousarray(np.concatenate(cols, axis=1))

    # thresholds, [128, 2*nblk] f32; column bk = t1 of gates bk*128..bk*128+127
    t1m = t1.reshape(nblk, 128).T.astype(np.float32)
    t2m = t2.reshape(nblk, 128).T.astype(np.float32)
    cst_np = np.ascontiguousarray(np.concatenate([t1m, t2m], axis=1))

    ident_np = np.eye(128, dtype=ml_dtypes.bfloat16)

    # doubled table
    xt = x8.T  # [n, B] view
    ntab = 2 * n + 2
    in_maps = []
    for k in range(ncores):
        sl = slice(k * bs, (k + 1) * bs)
        tabk = np.empty((ntab, bs), dtype=np.uint8)
        tabk[:n] = xt[:, sl]
        tabk[n : 2 * n] = 1 - tabk[:n]
        tabk[2 * n] = 1
        tabk[2 * n + 1] = 0
        in_maps.append(
            {"tab": tabk, "idxs": idxs_np, "cst": cst_np, "ident": ident_np}
        )
    return in_maps


# ---------------------------------------------------------------------------
# Entry point
# ---------------------------------------------------------------------------

_NC_CACHE = {}


def _get_nc(key=(BS, N, G, 8)):
    if key not in _NC_CACHE:
        _NC_CACHE[key] = build_nc(*key)
    return _NC_CACHE[key]


def kernel(x, gates, choices):
    in_maps = _prep_inputs(x, gates, choices)
    nc = _get_nc()
    res = run_bass_kernel_spmd(nc, in_maps, list(range(NCORES)))
    out = np.concatenate([res.results[k]["out"] for k in range(NCORES)], axis=0)
    return out.astype(bool)



# revision 2
# speedup vs baseline: 1.0723x; 1.0723x over previous
"""GateRow kernel for Trainium2 — 8 NeuronCores, SPMD gate-sharded, bit-packed.

v4: 2 gathered rows per gate; 6 "AND-form" blocks (2 stt ops each) +
2 "XOR-form" blocks (3 stt ops each) per core; per-partition int32 masks.

Problem: out[b, g] = gates[g, 2*x[b, c0[g]] + x[b, c1[g]]]
  x: [16384, 8192] bool, gates: [8192, 4] bool, choices: [8192, 2] int32.

Math: with u = (a or ~a) row and v in {b, ~b, a, ~a} rows of the doubled
packed table and per-gate masks in {0, -1}:
  AND-form:  f = ((u ^ mi) & v) ^ mo          (14 of 16 truth tables)
  XOR-form:  f = (u & v & mg) ^ (u & ma) ^ (v & mb)   (all 16)
XOR/XNOR gates (~12.5%) are placed in the XOR-form blocks (256 slots,
binomial tail makes overflow essentially impossible; asserted).
"""

import sys

for _p in ("/opt/trn_rl_repo", "/opt/pypackages"):
    if _p not in sys.path:
        sys.path.append(_p)

from contextlib import ExitStack

import numpy as np

import concourse.bass as bass
import concourse.bacc as bacc
import concourse.tile as tile
import concourse.mybir as mybir
from concourse.bass_utils import run_bass_kernel_spmd

B, N, G, NCORES = 16384, 8192, 8192, 8
GPC = G // NCORES          # 1024 gates per core
NBLK = GPC // 128          # 8 blocks of 128 gates
RB = B // 8                # 2048 packed bytes per row
RW = RB // 4               # 512 int32 words per row
NTAB = 2 * N               # x rows then ~x rows
NIDX = 2 * GPC             # 2 gathered rows per gate
NCALL = 4
BPC = NBLK // NCALL        # 2 blocks per gather call

# device block order (block slot -> form); XOR-form blocks mid-pipeline
# so the final gather group finishes with cheap 2-op blocks.
BLOCK_FORM = ["A", "A", "A", "X", "A", "X", "A", "A"]
XBLOCKS = [i for i, f in enumerate(BLOCK_FORM) if f == "X"]
ABLOCKS = [i for i, f in enumerate(BLOCK_FORM) if f == "A"]
# mask column layout: AND block -> 2 cols, XOR block -> 3 cols
_MCOL = {}
_c = 0
for _bk in range(NBLK):
    _MCOL[_bk] = _c
    _c += 2 if BLOCK_FORM[_bk] == "A" else 3
MCOLS = _c

# ---------------------------------------------------------------------------
# Classification.
#   AND-form reps: tt -> (uc, vc, mi, mo), f = ((u^mi)&v)^mo
#   XOR-form reps: tt -> (uc, vc, mg, ma, mb), f = (u&v&mg)^(u&ma)^(v&mb)
#   u = a^uc ; v = (b, ~b, a, ~a)[vc]
# ---------------------------------------------------------------------------


def _classify():
    and_rep = {}
    xor_rep = {}
    for tt in range(16):
        for uc in range(4):
            for vc in range(4):
                for mi in (0, 1):
                    for mo in (0, 1):
                        ok = all(
                            ((((a, 1 - a, b, 1 - b)[uc] ^ mi)
                              & (b, 1 - b, a, 1 - a)[vc]) ^ mo)
                            == ((tt >> (2 * a + b)) & 1)
                            for a in (0, 1)
                            for b in (0, 1)
                        )
                        if ok and tt not in and_rep:
                            and_rep[tt] = (uc, vc, mi, mo)
        for uc in range(2):
            for vc in range(4):
                for mg in (0, 1):
                    for ma in (0, 1):
                        for mb in (0, 1):
                            u_of = lambda a: a ^ uc
                            v_of = lambda a, b: (b, 1 - b, a, 1 - a)[vc]
                            ok = all(
                                ((u_of(a) & v_of(a, b) & mg)
                                 ^ (u_of(a) & ma) ^ (v_of(a, b) & mb))
                                == ((tt >> (2 * a + b)) & 1)
                                for a in (0, 1)
                                for b in (0, 1)
                            )
                            if ok and tt not in xor_rep:
                                xor_rep[tt] = (uc, vc, mg, ma, mb)
        assert tt in xor_rep, f"tt {tt} lacks XOR-form rep"
    return and_rep, xor_rep


_AND_REP, _XOR_REP = _classify()

# ---------------------------------------------------------------------------
# Device program
# ---------------------------------------------------------------------------


def _stt(nc, out, in0, scalar, in1, op0, op1):
    eng = nc.vector
    if isinstance(scalar, int):
        scalar_l = mybir.ImmediateValue(dtype=mybir.dt.int32, value=scalar)
    else:
        scalar_l = eng.lower_ap(scalar)
    return eng.add_instruction(
        mybir.InstTensorScalarPtr(
            name=eng.bass.get_next_instruction_name(),
            is_scalar_tensor_tensor=True,
            op0=op0,
            op1=op1,
            ins=[eng.lower_ap(in0), scalar_l, eng.lower_ap(in1)],
            outs=[eng.lower_ap(out)],
        )
    )


def build_nc(ncores=NCORES):
    nc = bacc.Bacc(
        "TRN2", target_bir_lowering=False, debug=False, num_devices=ncores,
    )
    tab = nc.dram_tensor("tab", [NTAB, RW], mybir.dt.int32, kind="ExternalInput")
    idxs = nc.dram_tensor(
        "idxs", [128, NIDX // 16], mybir.dt.int16, kind="ExternalInput"
    )
    msk = nc.dram_tensor("msk", [128, MCOLS], mybir.dt.int32, kind="ExternalInput")
    outd = nc.dram_tensor("out", [GPC, RW], mybir.dt.int32, kind="ExternalOutput")

    AL = mybir.AluOpType
    percall = NIDX // NCALL
    ccols = percall // 16

    with tile.TileContext(nc) as tc, ExitStack() as ctx:
        pconst = ctx.enter_context(tc.tile_pool(name="const", bufs=1))
        pg = ctx.enter_context(tc.tile_pool(name="gather", bufs=4))
        pt = ctx.enter_context(tc.tile_pool(name="tmp", bufs=4))
        po = ctx.enter_context(tc.tile_pool(name="osb", bufs=4))

        idx_t = pconst.tile([128, NIDX // 16], mybir.dt.int16)
        nc.sync.dma_start(idx_t[:], idxs[:])
        msk_t = pconst.tile([128, MCOLS], mybir.dt.int32)
        nc.sync.dma_start(msk_t[:], msk[:])
        z_t = pconst.tile([128, RW], mybir.dt.int32)
        nc.vector.memset(z_t[:], 0)

        for ci in range(NCALL):
            g_t = pg.tile([128, 2 * BPC, RW], mybir.dt.int32, tag="g")
            nc.gpsimd.dma_gather(
                g_t[:],
                tab[:],
                idx_t[:, ci * ccols : (ci + 1) * ccols],
                percall,
                percall,
                RW,
                single_packet=False,
            )
            for j in range(BPC):
                bk = ci * BPC + j
                u = g_t[:, 2 * j, :]
                v = g_t[:, 2 * j + 1, :]
                mc = _MCOL[bk]
                o_t = po.tile([128, RW], mybir.dt.int32, tag=f"o{j}")
                if BLOCK_FORM[bk] == "A":
                    t1 = pt.tile([128, RW], mybir.dt.int32, tag=f"t1_{j}")
                    _stt(nc, t1[:], u, msk_t[:, mc : mc + 1], v,
                         AL.bitwise_xor, AL.bitwise_and)
                    _stt(nc, o_t[:], t1[:], msk_t[:, mc + 1 : mc + 2], z_t[:],
                         AL.bitwise_xor, AL.bitwise_xor)
                else:
                    t1 = pt.tile([128, RW], mybir.dt.int32, tag=f"t1_{j}")
                    t2 = pt.tile([128, RW], mybir.dt.int32, tag=f"t2_{j}")
                    _stt(nc, t1[:], u, msk_t[:, mc : mc + 1], v,
                         AL.bitwise_and, AL.bitwise_and)
                    _stt(nc, t2[:], u, msk_t[:, mc + 1 : mc + 2], t1[:],
                         AL.bitwise_and, AL.bitwise_xor)
                    _stt(nc, o_t[:], v, msk_t[:, mc + 2 : mc + 3], t2[:],
                         AL.bitwise_and, AL.bitwise_xor)
                nc.sync.dma_start(outd[bk * 128 : (bk + 1) * 128, :], o_t[:])
    nc.compile()
    return nc


# ---------------------------------------------------------------------------
# Host-side gate placement + input prep
# ---------------------------------------------------------------------------


def _prep_inputs(x, gates, choices):
    x = np.asarray(x, dtype=bool)
    gates8 = np.asarray(gates, dtype=np.uint8)
    ch = np.asarray(choices, dtype=np.int64)

    xp = np.packbits(x, axis=0).T            # [N, RB] uint8
    tabu8 = np.empty((NTAB, RB), dtype=np.uint8)
    tabu8[:N] = xp
    tabu8[N:] = xp ^ 0xFF
    tab = np.ascontiguousarray(tabu8).view(np.int32)   # [NTAB, RW]

    tt_all = (gates8 << np.arange(4, dtype=np.uint8)).sum(axis=1).astype(np.int64)

    in_maps = []
    perm = np.empty(G, dtype=np.int64)       # perm[core*GPC + device_slot] = gate
    for c in range(NCORES):
        g0 = c * GPC
        tts = tt_all[g0 : g0 + GPC]
        is_x = np.array([tts[g] not in _AND_REP for g in range(GPC)])
        xg = np.where(is_x)[0]
        ag = np.where(~is_x)[0]
        assert len(xg) <= 128 * len(XBLOCKS), (
            f"core {c}: {len(xg)} XOR-form gates exceed capacity"
        )
        # fill XOR blocks with XOR gates then spill AND gates; AND blocks
        # take the rest
        xslots = [bk * 128 + p for bk in XBLOCKS for p in range(128)]
        aslots = [bk * 128 + p for bk in ABLOCKS for p in range(128)]
        fill_x = list(xg) + list(ag[: len(xslots) - len(xg)])
        fill_a = list(ag[len(xslots) - len(xg) :])
        assert len(fill_a) == len(aslots)

        urow = np.empty(GPC, dtype=np.int16)
        vrow = np.empty(GPC, dtype=np.int16)
        mcols = np.zeros((128, MCOLS), dtype=np.int32)

        def vrow_of(g, vc):
            return (ch[g0 + g, 1], N + ch[g0 + g, 1],
                    ch[g0 + g, 0], N + ch[g0 + g, 0])[vc]

        def urow_of(g, uc):
            return (ch[g0 + g, 0], N + ch[g0 + g, 0],
                    ch[g0 + g, 1], N + ch[g0 + g, 1])[uc]

        for slot, g in zip(xslots, fill_x):
            bk, p = divmod(slot, 128)
            uc, vc, mg, ma, mb = _XOR_REP[tts[g]]
            urow[slot] = urow_of(g, uc)
            vrow[slot] = vrow_of(g, vc)
            mc = _MCOL[bk]
            mcols[p, mc] = -mg
            mcols[p, mc + 1] = -ma
            mcols[p, mc + 2] = -mb
            perm[g0 + slot] = g0 + g
        for slot, g in zip(aslots, fill_a):
            bk, p = divmod(slot, 128)
            uc, vc, mi, mo = _AND_REP[tts[g]]
            urow[slot] = urow_of(g, uc)
            vrow[slot] = vrow_of(g, vc)
            mc = _MCOL[bk]
            mcols[p, mc] = -mi
            mcols[p, mc + 1] = -mo
            perm[g0 + slot] = g0 + g

        # gather idx layout: flat[i], i = col*128 + p, col = 2*bk + (0:u, 1:v)
        inter = np.empty((2 * NBLK, 128), dtype=np.int16)
        inter[0::2] = urow.reshape(NBLK, 128)
        inter[1::2] = vrow.reshape(NBLK, 128)
        flat = inter.reshape(-1)
        wrapped = flat.reshape(-1, 16).T
        idxs_np = np.ascontiguousarray(np.tile(wrapped, (8, 1)))
        in_maps.append({"tab": tab, "idxs": idxs_np, "msk": mcols})
    return in_maps, perm


# ---------------------------------------------------------------------------
# Entry point
# ---------------------------------------------------------------------------

_NC_CACHE = {}


def _get_nc():
    if "nc" not in _NC_CACHE:
        _NC_CACHE["nc"] = build_nc()
    return _NC_CACHE["nc"]


def kernel(x, gates, choices):
    in_maps, perm = _prep_inputs(x, gates, choices)
    nc = _get_nc()
    res = run_bass_kernel_spmd(nc, in_maps, list(range(NCORES)))
    packed = np.concatenate(
        [res.results[k]["out"] for k in range(NCORES)], axis=0
    )
    inv = np.empty(G, dtype=np.int64)
    inv[perm] = np.arange(G)
    bits = np.unpackbits(
        np.ascontiguousarray(packed).view(np.uint8), axis=1
    )
    return np.ascontiguousarray(bits[inv].T).astype(bool)


# revision 3
# speedup vs baseline: 1.1412x; 1.0642x over previous
"""GateRow kernel for Trainium2 — 8 NeuronCores, SPMD gate-sharded, bit-packed.

v6: like v4 (2 gathered rows/gate, AND-form + XOR-form blocks) but gates
that depend on at most one input (a, ~a, b, ~b, 0, 1 — ~37.5%) are placed
in two dedicated "U-form" blocks that gather only ONE row per gate,
cutting dma_gather descriptors 2048 -> 1792 per core and making the last
gather call tiny.

Problem: out[b, g] = gates[g, 2*x[b, c0[g]] + x[b, c1[g]]]
  x: [16384, 8192] bool, gates: [8192, 4] bool, choices: [8192, 2] int32.

Forms (u, v are rows of the doubled packed table; masks in {0, -1}):
  A:  f = ((u ^ mi) & v) ^ mo          2 stt ops   (all but XOR/XNOR)
  X:  f = (u & v & mg) ^ (u & ma) ^ (v & mb)   3 stt ops  (universal)
  U:  f = u                            0 stt ops, 1 gathered row
      (polarity/constness resolved by row choice; zeros/ones const rows)
"""

import sys

for _p in ("/opt/trn_rl_repo", "/opt/pypackages"):
    if _p not in sys.path:
        sys.path.append(_p)

from contextlib import ExitStack

import numpy as np

import concourse.bass as bass
import concourse.bacc as bacc
import concourse.tile as tile
import concourse.mybir as mybir
from concourse.bass_utils import run_bass_kernel_spmd

B, N, G, NCORES = 16384, 8192, 8192, 8
GPC = G // NCORES          # 1024 gates per core
NBLK = GPC // 128          # 8 blocks of 128 gates
RB = B // 8                # 2048 packed bytes per row
RW = RB // 4               # 512 int32 words per row
NTAB = 2 * N + 2           # x rows, ~x rows, zeros row, ones row

BLOCK_FORM = ["A", "A", "A", "A", "X", "X", "U", "U"]
CALL_BLOCKS = [[0, 1], [2, 3], [4, 5], [6, 7]]   # U,U last: smallest tail

_COLS_OF = {"A": 2, "X": 2, "U": 1}              # gather columns per block
_MASKS_OF = {"A": 2, "X": 3, "U": 0}
_COLOF = {}
_MCOL = {}
_c = _m = 0
for _bk in range(NBLK):
    _COLOF[_bk] = _c
    _MCOL[_bk] = _m
    _c += _COLS_OF[BLOCK_FORM[_bk]]
    _m += _MASKS_OF[BLOCK_FORM[_bk]]
NCOLS = _c                                        # 14 gather columns
MCOLS = _m
NIDX = NCOLS * 128                                # 1792 rows per core

# ---------------------------------------------------------------------------
# Classification.
# ---------------------------------------------------------------------------


def _classify():
    and_rep = {}
    xor_rep = {}
    u_rep = {}
    for tt in range(16):
        for uc in range(4):
            for vc in range(4):
                for mi in (0, 1):
                    for mo in (0, 1):
                        ok = all(
                            ((((a, 1 - a, b, 1 - b)[uc] ^ mi)
                              & (b, 1 - b, a, 1 - a)[vc]) ^ mo)
                            == ((tt >> (2 * a + b)) & 1)
                            for a in (0, 1)
                            for b in (0, 1)
                        )
                        if ok and tt not in and_rep:
                            and_rep[tt] = (uc, vc, mi, mo)
        for uc in range(2):
            for vc in range(4):
                for mg in (0, 1):
                    for ma in (0, 1):
                        for mb in (0, 1):
                            ok = all(
                                (((a ^ uc) & (b, 1 - b, a, 1 - a)[vc] & mg)
                                 ^ ((a ^ uc) & ma)
                                 ^ ((b, 1 - b, a, 1 - a)[vc] & mb))
                                == ((tt >> (2 * a + b)) & 1)
                                for a in (0, 1)
                                for b in (0, 1)
                            )
                            if ok and tt not in xor_rep:
                                xor_rep[tt] = (uc, vc, mg, ma, mb)
        # U-form: f is a literal or constant; rc codes the row:
        # 0: c0, 1: N+c0, 2: c1, 3: N+c1, 4: zeros row, 5: ones row
        for rc in range(6):
            ok = all(
                ((a, 1 - a, b, 1 - b, 0, 1)[rc] == ((tt >> (2 * a + b)) & 1))
                for a in (0, 1)
                for b in (0, 1)
            )
            if ok and tt not in u_rep:
                u_rep[tt] = (rc,)
        assert tt in xor_rep, f"tt {tt} lacks XOR-form rep"
    return and_rep, xor_rep, u_rep


_AND_REP, _XOR_REP, _U_REP = _classify()

# ---------------------------------------------------------------------------
# Device program
# ---------------------------------------------------------------------------


def _stt(nc, out, in0, scalar, in1, op0, op1):
    eng = nc.vector
    if isinstance(scalar, int):
        scalar_l = mybir.ImmediateValue(dtype=mybir.dt.int32, value=scalar)
    else:
        scalar_l = eng.lower_ap(scalar)
    return eng.add_instruction(
        mybir.InstTensorScalarPtr(
            name=eng.bass.get_next_instruction_name(),
            is_scalar_tensor_tensor=True,
            op0=op0,
            op1=op1,
            ins=[eng.lower_ap(in0), scalar_l, eng.lower_ap(in1)],
            outs=[eng.lower_ap(out)],
        )
    )


def build_nc(ncores=NCORES):
    nc = bacc.Bacc(
        "TRN2", target_bir_lowering=False, debug=False, num_devices=ncores,
    )
    tab = nc.dram_tensor("tab", [NTAB, RW], mybir.dt.int32, kind="ExternalInput")
    idxs = nc.dram_tensor(
        "idxs", [128, NIDX // 16], mybir.dt.int16, kind="ExternalInput"
    )
    msk = nc.dram_tensor("msk", [128, MCOLS], mybir.dt.int32, kind="ExternalInput")
    outd = nc.dram_tensor("out", [GPC, RW], mybir.dt.int32, kind="ExternalOutput")

    AL = mybir.AluOpType

    with tile.TileContext(nc) as tc, ExitStack() as ctx:
        pconst = ctx.enter_context(tc.tile_pool(name="const", bufs=1))
        pg = ctx.enter_context(tc.tile_pool(name="gather", bufs=4))
        pt = ctx.enter_context(tc.tile_pool(name="tmp", bufs=4))
        po = ctx.enter_context(tc.tile_pool(name="osb", bufs=4))

        idx_t = pconst.tile([128, NIDX // 16], mybir.dt.int16)
        nc.sync.dma_start(idx_t[:], idxs[:])
        msk_t = pconst.tile([128, MCOLS], mybir.dt.int32)
        nc.sync.dma_start(msk_t[:], msk[:])
        z_t = pconst.tile([128, RW], mybir.dt.int32)
        nc.vector.memset(z_t[:], 0)

        for blist in CALL_BLOCKS:
            ncols = sum(_COLS_OF[BLOCK_FORM[bk]] for bk in blist)
            percall = ncols * 128
            c0 = _COLOF[blist[0]]
            g_t = pg.tile([128, ncols, RW], mybir.dt.int32, tag=f"g{ncols}")
            nc.gpsimd.dma_gather(
                g_t[:],
                tab[:],
                idx_t[:, c0 * 8 : c0 * 8 + percall // 16],
                percall,
                percall,
                RW,
                single_packet=False,
            )
            for bk in blist:
                gc = _COLOF[bk] - c0
                mc = _MCOL[bk]
                form = BLOCK_FORM[bk]
                o_t = po.tile([128, RW], mybir.dt.int32, tag=f"o{bk % 4}")
                if form == "A":
                    u = g_t[:, gc, :]
                    v = g_t[:, gc + 1, :]
                    t1 = pt.tile([128, RW], mybir.dt.int32, tag=f"t1_{bk % 4}")
                    _stt(nc, t1[:], u, msk_t[:, mc : mc + 1], v,
                         AL.bitwise_xor, AL.bitwise_and)
                    _stt(nc, o_t[:], t1[:], msk_t[:, mc + 1 : mc + 2], z_t[:],
                         AL.bitwise_xor, AL.bitwise_xor)
                elif form == "X":
                    u = g_t[:, gc, :]
                    v = g_t[:, gc + 1, :]
                    t1 = pt.tile([128, RW], mybir.dt.int32, tag=f"t1_{bk % 4}")
                    t2 = pt.tile([128, RW], mybir.dt.int32, tag=f"t2_{bk % 4}")
                    _stt(nc, t1[:], u, msk_t[:, mc : mc + 1], v,
                         AL.bitwise_and, AL.bitwise_and)
                    _stt(nc, t2[:], u, msk_t[:, mc + 1 : mc + 2], t1[:],
                         AL.bitwise_and, AL.bitwise_xor)
                    _stt(nc, o_t[:], v, msk_t[:, mc + 2 : mc + 3], t2[:],
                         AL.bitwise_and, AL.bitwise_xor)
                else:  # U: passthrough of the gathered row
                    nc.sync.dma_start(
                        outd[bk * 128 : (bk + 1) * 128, :], g_t[:, gc, :]
                    )
                    continue
                nc.sync.dma_start(outd[bk * 128 : (bk + 1) * 128, :], o_t[:])
    nc.compile()
    return nc


# ---------------------------------------------------------------------------
# Host-side gate placement + input prep
# ---------------------------------------------------------------------------


def _prep_inputs(x, gates, choices):
    x = np.asarray(x, dtype=bool)
    gates8 = np.asarray(gates, dtype=np.uint8)
    ch = np.asarray(choices, dtype=np.int64)

    xp = np.packbits(x, axis=0).T
    tabu8 = np.empty((NTAB, RB), dtype=np.uint8)
    tabu8[:N] = xp
    tabu8[N : 2 * N] = xp ^ 0xFF
    tabu8[2 * N] = 0x00
    tabu8[2 * N + 1] = 0xFF
    tab = np.ascontiguousarray(tabu8).view(np.int32)

    tt_all = (gates8 << np.arange(4, dtype=np.uint8)).sum(axis=1).astype(np.int64)

    ublocks = [bk for bk in range(NBLK) if BLOCK_FORM[bk] == "U"]
    xblocks = [bk for bk in range(NBLK) if BLOCK_FORM[bk] == "X"]
    ablocks = [bk for bk in range(NBLK) if BLOCK_FORM[bk] == "A"]

    in_maps = []
    perm = np.empty(G, dtype=np.int64)
    for c in range(NCORES):
        g0 = c * GPC
        tts = tt_all[g0 : g0 + GPC]
        is_u = np.array([tts[g] in _U_REP for g in range(GPC)])
        is_x = np.array([tts[g] not in _AND_REP for g in range(GPC)])
        ug = np.where(is_u)[0]
        xg = np.where(is_x)[0]
        ag = np.where(~is_u & ~is_x)[0]
        ucap = 128 * len(ublocks)
        xcap = 128 * len(xblocks)
        assert len(ug) >= ucap, f"core {c}: too few 1-input gates ({len(ug)})"
        assert len(xg) <= xcap, f"core {c}: too many XOR gates ({len(xg)})"
        # U blocks take 1-input gates; X blocks take XOR gates then spill;
        # A blocks take the rest (AND/OR + leftover 1-input gates).
        fill_u = list(ug[:ucap])
        rest = list(ug[ucap:]) + list(ag)
        fill_x = list(xg) + rest[: xcap - len(xg)]
        fill_a = rest[xcap - len(xg) :]

        rows = np.zeros(NCOLS * 128, dtype=np.int16)   # gather column data
        mcols = np.zeros((128, MCOLS), dtype=np.int32)

        def c0_(g):
            return ch[g0 + g, 0]

        def c1_(g):
            return ch[g0 + g, 1]

        def urow_of(g, uc):
            return (c0_(g), N + c0_(g), c1_(g), N + c1_(g))[uc]

        def vrow_of(g, vc):
            return (c1_(g), N + c1_(g), c0_(g), N + c0_(g))[vc]

        for blocks, fill in ((ublocks, fill_u), (xblocks, fill_x),
                             (ablocks, fill_a)):
            for i, g in enumerate(fill):
                bk = blocks[i // 128]
                p = i % 128
                slot = bk * 128 + p
                col = _COLOF[bk]
                mc = _MCOL[bk]
                form = BLOCK_FORM[bk]
                tt = tts[g]
                if form == "U":
                    (rc,) = _U_REP[tt]
                    rows[col * 128 + p] = (
                        c0_(g), N + c0_(g), c1_(g), N + c1_(g),
                        2 * N, 2 * N + 1,
                    )[rc]
                elif form == "X":
                    uc, vc, mg, ma, mb = _XOR_REP[tt]
                    rows[col * 128 + p] = c0_(g) + uc * N
                    rows[(col + 1) * 128 + p] = vrow_of(g, vc)
                    mcols[p, mc] = -mg
                    mcols[p, mc + 1] = -ma
                    mcols[p, mc + 2] = -mb
                else:
                    uc, vc, mi, mo = _AND_REP[tt]
                    rows[col * 128 + p] = urow_of(g, uc)
                    rows[(col + 1) * 128 + p] = vrow_of(g, vc)
                    mcols[p, mc] = -mi
                    mcols[p, mc + 1] = -mo
                perm[g0 + slot] = g0 + g

        wrapped = rows.reshape(-1, 16).T
        idxs_np = np.ascontiguousarray(np.tile(wrapped, (8, 1)))
        in_maps.append({"tab": tab, "idxs": idxs_np, "msk": mcols})
    return in_maps, perm


# ---------------------------------------------------------------------------
# Entry point
# ---------------------------------------------------------------------------

_NC_CACHE = {}


def _get_nc():
    if "nc" not in _NC_CACHE:
        _NC_CACHE["nc"] = build_nc()
    return _NC_CACHE["nc"]


def kernel(x, gates, choices):
    in_maps, perm = _prep_inputs(x, gates, choices)
    nc = _get_nc()
    res = run_bass_kernel_spmd(nc, in_maps, list(range(NCORES)))
    packed = np.concatenate(
        [res.results[k]["out"] for k in range(NCORES)], axis=0
    )
    inv = np.empty(G, dtype=np.int64)
    inv[perm] = np.arange(G)
    bits = np.unpackbits(
        np.ascontiguousarray(packed).view(np.uint8), axis=1
    )
    return np.ascontiguousarray(bits[inv].T).astype(bool)
